# revision 3
# baseline (speedup 1.0000x reference)
"""Trainium2 Bass kernel for BaselineNet (quantized 3D CNN), 8-core data parallel.

Pipeline per core (128 images, sets of 4):
  conv1: on-device im2col via 15 strided DMAs -> x60[(kd,kh,j), (do,ho,w)],
         block-diagonal [60,128] lhsT computes 4 images per matmul,
         5 accumulating matmuls (kw) x 7 PSUM chunks.
  conv2: c96[(kd,ci), ...] built with 12 SBUF DMAs, 9 accumulating matmuls
         per (img, 2-plane chunk), 4 images packed into one [128,288] PSUM tile.
  pool:  2x2x2 max on DVE; +b2 (b1 pre-folded through conv2); fp16 features
         stored to DRAM [img, 6912].
  fc1:   54 xbar-transposed loads [128 feat, 128 img] + accumulating matmuls;
         weights shipped as int4-packed codes (quant scale folded into fc2/bias).
  fc2 + softmax on chip.

Transfer budget per call (~21MB over the axon tunnel; the wall-clock floor):
  x as int8 (per-tensor scale folded into conv1 weights)  16MB
  fc1 weights int4-packed                                  3.5MB
  conv weights fp16 + biases                               ~1.2MB
"""

import numpy as np

import jax as _jax

try:
    _jax.config.update("jax_compilation_cache_dir", "/tmp/jax_cc_cache")
    _jax.config.update("jax_persistent_cache_min_compile_time_secs", 0.0)
    _jax.config.update("jax_persistent_cache_min_entry_size_bytes", 0)
except Exception:
    pass

import concourse.bass as bass
import concourse.bacc as bacc_mod
import concourse.mybir as mybir
from concourse.tile import TileContext
from concourse.ap import AP
from concourse.bass_utils import run_bass_kernel_spmd

F16 = mybir.dt.float16
F32 = mybir.dt.float32
I8 = mybir.dt.int8
U8 = mybir.dt.uint8

N_CORES = 8
B_CORE = 128
G = 4                  # images per set
N_SETS = B_CORE // G   # 32

# conv1 geometry: in (32,16,32), k=(5,3,5), s=(2,1,2) -> out (14,14,14)
P1 = 14 * 14 * 14      # 2744
CV1_CHUNK = 392        # 2 do-planes
CV1_NCHUNK = 7
# conv2 geometry: k=3 -> out (12,12,12)
D2 = 12
C96_FREE = 12 * 14 * 14   # 2352 per image
CV2_CHUNK = 288           # 2 d-planes * 144
CV2_NCHUNK = 6
POOL_F = 216
FDIM = 6912
FC_NCHUNK = 54

# blob layout, in f16 elements: int4 fc1 codes (bitcast u8), conv weights,
# fc2 weights, biases-as-f16 packed [128, 6] = (b2r, bf1c, bf2f)
O_W1T = FDIM // 4 * 128        # 221184
O_W2T = O_W1T + 38400          # 259584
O_WF2 = O_W2T + 27648          # 287232
O_BIAS = O_WF2 + 512           # 287744
BLOB_F16 = O_BIAS + 768        # 288512


def _fake_quant(w):
    n = 7.0
    w = np.asarray(w, np.float32)
    scale = np.max(np.abs(w)) / n
    q = np.clip(np.round(w / scale), -n, n) * scale
    return q.astype(np.float32)


def _build_nc():
    nc = bacc_mod.Bacc(None, target_bir_lowering=False)
    x_d = nc.declare_dram_parameter("x", [B_CORE, 32, 16, 32], I8, isOutput=False)
    # all weights/biases in ONE param: each extra sharded device_put costs
    # ~60-90ms of fixed tunnel overhead, so 6 small arrays -> 1 blob
    blob_d = nc.declare_dram_parameter("blob", [BLOB_F16], F16, isOutput=False)
    out_d = nc.declare_dram_parameter("out", [B_CORE, 4], F32, isOutput=True)
    f_dram = nc.dram_tensor("fbuf", [B_CORE, FDIM], F16)

    with TileContext(nc) as tc:
        with (
            tc.tile_pool(name="wpool", bufs=1) as wpool,
            tc.tile_pool(name="xpool", bufs=2) as xpool,
            tc.tile_pool(name="c1pool", bufs=2) as c1pool,
            tc.tile_pool(name="c96pool", bufs=2) as c96pool,
            tc.tile_pool(name="ppool", bufs=2) as ppool,
            tc.tile_pool(name="scratch", bufs=2) as scratch,
            tc.tile_pool(name="ps1", bufs=2, space="PSUM") as ps1pool,
            tc.tile_pool(name="ps2", bufs=3, space="PSUM") as ps2pool,
            tc.tile_pool(name="fpool", bufs=3) as fpool,
            tc.tile_pool(name="psf", bufs=1, space="PSUM") as psfpool,
            tc.tile_pool(name="ps4", bufs=1, space="PSUM") as ps4pool,
        ):
            # ---- weights / constants, loaded once (all slices of blob_d)
            w1sb = wpool.tile([60, 5 * 128], F16, tag="w1sb")
            nc.sync.dma_start(
                out=w1sb[:],
                in_=blob_d[O_W1T : O_W1T + 38400].rearrange("(p f) -> p f", p=60),
            )
            w2sb = wpool.tile([96, 9 * 32], F16, tag="w2sb")
            nc.sync.dma_start(
                out=w2sb[:],
                in_=blob_d[O_W2T : O_W2T + 27648].rearrange("(p f) -> p f", p=96),
            )
            # fc1 weights: int4-packed codes -> fp16 (two tensor_scalar unpacks)
            wf1p = wpool.tile([128, FDIM // 2], U8, tag="wf1p")
            nc.sync.dma_start(
                out=wf1p[:],
                in_=blob_d[0 : FDIM // 4 * 128].bitcast(U8).rearrange(
                    "(p f) -> p f", p=128
                ),
            )
            wf1f = wpool.tile([128, FDIM], F16, tag="wf1f")
            wf1v = wf1f.rearrange("p (i two) -> p i two", two=2)
            # (walrus rejects bitwise+arith in one tensor_scalar: split)
            u0 = wpool.tile([128, FDIM // 2], U8, tag="u0")
            nc.vector.tensor_scalar(
                u0[:], wf1p[:], 15, None, op0=mybir.AluOpType.bitwise_and
            )
            nc.vector.tensor_scalar_add(wf1v[:, :, 0], u0[:], -8.0)
            u1 = wpool.tile([128, FDIM // 2], U8, tag="u1")
            nc.vector.tensor_scalar(
                u1[:], wf1p[:], 4, None, op0=mybir.AluOpType.logical_shift_right
            )
            nc.vector.tensor_scalar_add(wf1v[:, :, 1], u1[:], -8.0)
            wf2t = wpool.tile([128, 4], F16, tag="wf2t")
            nc.sync.dma_start(
                out=wf2t[:],
                in_=blob_d[O_WF2 : O_WF2 + 512].rearrange("(p f) -> p f", p=128),
            )
            # biases travel as f16, converted to f32 tiles on chip
            bh = wpool.tile([128, 6], F16, tag="bh")
            nc.sync.dma_start(
                out=bh[:],
                in_=blob_d[O_BIAS : O_BIAS + 768].rearrange("(p f) -> p f", p=128),
            )
            b2r = wpool.tile([128, 1], F32, tag="b2r")
            nc.vector.tensor_copy(b2r[:], bh[:, 0:1])
            bf1c = wpool.tile([128, 1], F32, tag="bf1c")
            nc.vector.tensor_copy(bf1c[:], bh[:, 1:2])
            bf2f = wpool.tile([128, 4], F32, tag="bf2f")
            nc.vector.tensor_copy(bf2f[:], bh[:, 2:6])
            # preload ACT exp LUT so the final Exp carries no table-DMA wait
            warm = wpool.tile([1, 1], F32, tag="warm")
            nc.scalar.activation(
                warm[:], b2r[0:1, :], mybir.ActivationFunctionType.Exp
            )

            for s in range(N_SETS):
                b0 = G * s
                # ---- conv1 im2col: x60[(kd,kh),(j), (do, 14h x 32w)], 15 DMAs
                x60i = xpool.tile([60, 14 * 448], I8, tag="x60i")
                x60p = x60i.rearrange("(t j) f -> t j f", j=G)
                for kd in range(5):
                    for kh in range(3):
                        t = kd * 3 + kh
                        src = AP(
                            x_d,
                            b0 * 16384 + kd * 512 + kh * 32,
                            [[16384, G], [1024, 14], [1, 448]],
                        )
                        nc.sync.dma_start(out=x60p[t], in_=src)
                x60 = xpool.tile([60, 14 * 448], F16, tag="x60")
                nc.vector.tensor_copy(x60[:], x60i[:])

                # ---- conv1: 5 accumulating matmuls (kw) x 7 chunks, 4 img each
                xv = x60.rearrange("p (do ho w) -> p do ho w", do=14, ho=14, w=32)
                c1 = c1pool.tile([128, P1], F16, tag="c1")
                for ch in range(CV1_NCHUNK):
                    ps1 = ps1pool.tile([128, CV1_CHUNK], F32, tag="ps1")
                    for kw in range(5):
                        rhs = xv[:, 2 * ch : 2 * ch + 2, :, kw : kw + 27 : 2]
                        nc.tensor.matmul(
                            ps1[:], w1sb[:, kw * 128 : (kw + 1) * 128], rhs,
                            start=(kw == 0), stop=(kw == 4),
                        )
                    nc.vector.tensor_copy(
                        c1[:, ch * CV1_CHUNK : (ch + 1) * CV1_CHUNK], ps1[:]
                    )

                # ---- conv2 im2col: c96[(kd,ci), (j, d', h, w)] via 12 DMAs
                c96 = c96pool.tile([96, G * C96_FREE], F16, tag="c96")
                c96r = c96.rearrange("q (j f) -> q j f", j=G)
                c1r = c1.rearrange("p (d hw) -> p d hw", d=14, hw=196)
                for kd in range(3):
                    for j in range(G):
                        nc.sync.dma_start(
                            out=c96r[32 * kd : 32 * kd + 32, j],
                            in_=c1r[32 * j : 32 * j + 32, kd : kd + 12, :],
                        )

                # ---- conv2 matmuls (4 img share a PSUM tile) + maxpool
                c96v = c96.rearrange(
                    "q (j d h w) -> q j d h w", j=G, d=D2, h=14, w=14
                )
                pall = ppool.tile([128, POOL_F], F32, tag="pall")
                for t in range(CV2_NCHUNK):
                    ps2 = ps2pool.tile([128, CV2_CHUNK], F32, tag="ps2")
                    for j in range(G):
                        for kk in range(9):
                            kh, kw = kk // 3, kk % 3
                            rhs = c96v[:, j, 2 * t : 2 * t + 2,
                                       kh : kh + D2, kw : kw + D2]
                            nc.tensor.matmul(
                                ps2[32 * j : 32 * j + 32, :],
                                w2sb[:, kk * 32 : (kk + 1) * 32], rhs,
                                start=(kk == 0), stop=(kk == 8),
                                tile_position=(0, 32 * j),
                            )
                    # maxpool 2x2x2 on [128, (2,12,12)] -> [128, 36]
                    # (DVE may read only ONE input from PSUM: copy evens, max odds)
                    pv = ps2.rearrange("p (d h w) -> p d h w", d=2, h=12, w=12)
                    t1 = scratch.tile([128, 144], F32, tag="t1")
                    t1v = t1.rearrange("p (d h w) -> p d h w", d=2, h=12, w=6)
                    nc.vector.tensor_copy(t1v[:], pv[:, :, :, 0::2])
                    nc.vector.tensor_max(t1v[:], t1v[:], pv[:, :, :, 1::2])
                    t2 = scratch.tile([128, 72], F32, tag="t2")
                    t2v = t2.rearrange("p (d h w) -> p d h w", d=2, h=6, w=6)
                    nc.vector.tensor_max(t2v[:], t1v[:, :, 0::2, :], t1v[:, :, 1::2, :])
                    nc.vector.tensor_max(
                        pall[:, t * 36 : (t + 1) * 36], t2[:, 0:36], t2[:, 36:72]
                    )
                # bias b2 (post-pool equivalent) + cast fp16
                psb = scratch.tile([128, POOL_F], F16, tag="psb")
                nc.vector.tensor_scalar_add(psb[:], pall[:], b2r[:])
                # store features [ (j,co), 216 ] -> f_dram[4 imgs, 6912]
                for j in range(G):
                    nc.sync.dma_start(
                        out=f_dram[b0 + j : b0 + j + 1, :],
                        in_=psb[32 * j : 32 * j + 32, :],
                    )

            # ---- fc1: K=6912 in 54 chunks via xbar-transposed loads
            psf = psfpool.tile([128, 128], F32, tag="psf")
            for c in range(FC_NCHUNK):
                fcc = fpool.tile([128, 128], F16, tag="fcc")
                nc.sync.dma_start(
                    out=fcc[:], in_=f_dram[:, 128 * c : 128 * (c + 1)],
                    transpose=True,
                )
                nc.tensor.matmul(
                    psf[:], wf1f[:, 128 * c : 128 * (c + 1)], fcc[:],
                    start=(c == 0), stop=(c == FC_NCHUNK - 1),
                )
            # relu(s1 + bf1/s) -> A1 [128(out_f), 128(img)] fp16
            s1t = fpool.tile([128, 128], F32, tag="s1t")
            nc.vector.tensor_scalar_add(s1t[:], psf[:], bf1c[:])
            a1 = fpool.tile([128, 128], F16, tag="a1")
            nc.vector.tensor_scalar_max(a1[:], s1t[:], 0.0)
            # fc2 (wf2 carries the fc1 quant scale)
            ps4 = ps4pool.tile([128, 4], F32, tag="ps4")
            nc.tensor.matmul(ps4[:], a1[:], wf2t[:], start=True, stop=True)
            s2 = scratch.tile([128, 4], F32, tag="s2")
            nc.vector.tensor_add(s2[:], ps4[:], bf2f[:])
            # softmax over free dim (4)
            nmax = scratch.tile([128, 1], F32, tag="nmax")
            nc.vector.reduce_max(
                out=nmax[:], in_=s2[:], axis=mybir.AxisListType.X, negate=True
            )
            ex = scratch.tile([128, 4], F32, tag="ex")
            esum = scratch.tile([128, 1], F32, tag="esum")
            nc.scalar.activation(
                ex[:], s2[:], mybir.ActivationFunctionType.Exp,
                bias=nmax[:], accum_out=esum[:],
            )
            rec = scratch.tile([128, 1], F32, tag="rec")
            nc.vector.reciprocal(rec[:], esum[:])
            outt = scratch.tile([128, 4], F32, tag="outt")
            nc.vector.tensor_scalar_mul(outt[:], ex[:], rec[:])
            nc.sync.dma_start(out=out_d[:], in_=outt[:])

    nc.compile()
    return nc


_CACHED = {}


def _host_prep(x, w1, b1, w2, b2, wf1, bf1, wf2, bf2):
    q1 = _fake_quant(w1)
    q2 = _fake_quant(w2)
    qf2 = _fake_quant(wf2)
    # fc1 weights as int4 codes; scale folded into wf2 / bf1
    sf1 = float(np.max(np.abs(np.asarray(wf1, np.float32))) / 7.0)
    kf1 = np.clip(
        np.round(np.asarray(wf1, np.float32) / sf1), -7, 7
    ).astype(np.int8)  # [128, 6912]

    # x as int8, per-tensor scale folded into conv1 weights
    Xf = np.asarray(x, np.float32)[:, 0]
    sx = float(np.max(np.abs(Xf)) / 127.0)
    Xi = np.clip(np.round(Xf * (1.0 / sx)), -127, 127).astype(np.int8)

    # conv1 weights: [60, 5*128], block-diag over 4 images, per-kw slices
    # partition order (t, j) with t = kd*3+kh
    w1t = np.zeros((5, 60, 128), np.float32)
    for kw in range(5):
        for kd in range(5):
            for kh in range(3):
                t = kd * 3 + kh
                for j in range(G):
                    w1t[kw, t * G + j, j * 32 : (j + 1) * 32] = q1[:, 0, kd, kh, kw]
    w1t = np.ascontiguousarray(
        (w1t * sx).transpose(1, 0, 2).reshape(60, 5 * 128)
    ).astype(np.float16)

    # conv2 weights: [96=(kd,ci), 9*32=(kk,co)]
    W2T = np.empty((9, 96, 32), np.float32)
    for kh in range(3):
        for kw in range(3):
            for kd in range(3):
                W2T[kh * 3 + kw, kd * 32 : (kd + 1) * 32, :] = q2[:, :, kd, kh, kw].T
    W2T = np.ascontiguousarray(
        W2T.transpose(1, 0, 2).reshape(96, 288)
    ).astype(np.float16)

    # fc1 int4 codes laid out [p, (chunk, out)] with p = feature-within-chunk,
    # then packed two-per-byte (lo nibble = even column, offset +8)
    wf1i = np.ascontiguousarray(
        kf1.T.reshape(FC_NCHUNK, 128, 128).transpose(1, 0, 2).reshape(128, FDIM)
    )
    lo = (wf1i[:, 0::2] + 8).astype(np.uint8)
    hi = (wf1i[:, 1::2] + 8).astype(np.uint8)
    wf1p = (lo | (hi << 4)).astype(np.uint8)

    wf2t = np.ascontiguousarray(qf2.T * sf1).astype(np.float16)  # [128, 4]

    b2p = np.asarray(b2, np.float32) + q2.sum(axis=(2, 3, 4)) @ np.asarray(
        b1, np.float32
    )
    b2r = np.tile(b2p, G)[:, None].astype(np.float32)
    bf1c = (np.asarray(bf1, np.float32) / sf1)[:, None]
    bf2f = np.tile(np.asarray(bf2, np.float32)[None, :], (128, 1))

    blob = np.empty(BLOB_F16, np.float16)
    blob[0:O_W1T] = wf1p.reshape(-1).view(np.float16)
    blob[O_W1T : O_W1T + 38400] = w1t.reshape(-1)
    blob[O_W2T : O_W2T + 27648] = W2T.reshape(-1)
    blob[O_WF2 : O_WF2 + 512] = wf2t.reshape(-1)
    blob[O_BIAS : O_BIAS + 768] = np.concatenate(
        [b2r, bf1c, bf2f], axis=1
    ).astype(np.float16).reshape(-1)
    return Xi, blob


def kernel(x, w1, b1, w2, b2, wf1, bf1, wf2, bf2):
    Xi, blob = _host_prep(x, w1, b1, w2, b2, wf1, bf1, wf2, bf2)
    if "nc" not in _CACHED:
        _CACHED["nc"] = _build_nc()
    nc = _CACHED["nc"]
    in_maps = []
    for c in range(N_CORES):
        in_maps.append({
            "x": Xi[c * B_CORE : (c + 1) * B_CORE],
            "blob": blob,
        })
    res = run_bass_kernel_spmd(nc, in_maps, list(range(N_CORES)))
    outs = [np.asarray(r["out"], np.float32) for r in res.results]
    return np.concatenate(outs, axis=0)


# revision 4
# speedup vs baseline: 1.2145x; 1.2145x over previous
"""Trainium2 Bass kernel for BaselineNet (quantized 3D CNN), 8-core data parallel.

Pipeline per core (128 images, sets of 4):
  conv1: on-device im2col via 15 strided DMAs -> x60[(kd,kh,j), (do,ho,w)],
         block-diagonal [60,128] lhsT computes 4 images per matmul,
         5 accumulating matmuls (kw) x 7 PSUM chunks.
  conv2: c96[(kd,ci), ...] built with 12 SBUF DMAs, 9 accumulating matmuls
         per (img, 2-plane chunk), 4 images packed into one [128,288] PSUM tile.
  pool:  2x2x2 max on DVE; +b2 (b1 pre-folded through conv2); fp16 features
         stored to DRAM [img, 6912].
  fc1:   54 xbar-transposed loads [128 feat, 128 img] + accumulating matmuls;
         weights shipped as int4-packed codes (quant scale folded into fc2/bias).
  fc2 + softmax on chip.

Transfer budget per call (~21MB over the axon tunnel; the wall-clock floor):
  x as int8 (per-tensor scale folded into conv1 weights)  16MB
  fc1 weights int4-packed                                  3.5MB
  conv weights fp16 + biases                               ~1.2MB
"""

import numpy as np

import jax as _jax

try:
    _jax.config.update("jax_compilation_cache_dir", "/tmp/jax_cc_cache")
    _jax.config.update("jax_persistent_cache_min_compile_time_secs", 0.0)
    _jax.config.update("jax_persistent_cache_min_entry_size_bytes", 0)
except Exception:
    pass

import concourse.bass as bass
import concourse.bacc as bacc_mod
import concourse.mybir as mybir
from concourse.tile import TileContext
from concourse.ap import AP
from concourse.bass_utils import run_bass_kernel_spmd

F16 = mybir.dt.float16
F32 = mybir.dt.float32
I8 = mybir.dt.int8
U8 = mybir.dt.uint8

N_CORES = 8
B_CORE = 128
G = 4                  # images per set
N_SETS = B_CORE // G   # 32

# conv1 geometry: in (32,16,32), k=(5,3,5), s=(2,1,2) -> out (14,14,14)
P1 = 14 * 14 * 14      # 2744
CV1_CHUNK = 392        # 2 do-planes
CV1_NCHUNK = 7
# conv2 geometry: k=3 -> out (12,12,12)
D2 = 12
C96_FREE = 12 * 14 * 14   # 2352 per image
CV2_CHUNK = 288           # 2 d-planes * 144
CV2_NCHUNK = 6
POOL_F = 216
FDIM = 6912
FC_NCHUNK = 54

# blob layout, in f16 elements: int4 fc1 codes (bitcast u8), conv weights,
# fc2 weights, biases-as-f16 packed [128, 6] = (b2r, bf1c, bf2f)
O_W1T = FDIM // 4 * 128        # 221184
O_W2T = O_W1T + 38400          # 259584
O_WF2 = O_W2T + 27648          # 287232
O_BIAS = O_WF2 + 512           # 287744
BLOB_F16 = O_BIAS + 768        # 288512


def _fake_quant(w):
    n = 7.0
    w = np.asarray(w, np.float32)
    scale = np.max(np.abs(w)) / n
    q = np.clip(np.round(w / scale), -n, n) * scale
    return q.astype(np.float32)


def _build_nc():
    nc = bacc_mod.Bacc(None, target_bir_lowering=False)
    x_d = nc.declare_dram_parameter("x", [B_CORE, 32, 16, 32], I8, isOutput=False)
    # all weights/biases in ONE param: each extra sharded device_put costs
    # ~60-90ms of fixed tunnel overhead, so 6 small arrays -> 1 blob
    blob_d = nc.declare_dram_parameter("blob", [BLOB_F16], F16, isOutput=False)
    out_d = nc.declare_dram_parameter("out", [B_CORE, 4], F32, isOutput=True)
    f_dram = nc.dram_tensor("fbuf", [B_CORE, FDIM], F16)

    with TileContext(nc) as tc:
        with (
            tc.tile_pool(name="wpool", bufs=1) as wpool,
            tc.tile_pool(name="xpool", bufs=2) as xpool,
            tc.tile_pool(name="c1pool", bufs=2) as c1pool,
            tc.tile_pool(name="c96pool", bufs=2) as c96pool,
            tc.tile_pool(name="ppool", bufs=2) as ppool,
            tc.tile_pool(name="scratch", bufs=2) as scratch,
            tc.tile_pool(name="ps1", bufs=2, space="PSUM") as ps1pool,
            tc.tile_pool(name="ps2", bufs=3, space="PSUM") as ps2pool,
            tc.tile_pool(name="fpool", bufs=3) as fpool,
            tc.tile_pool(name="psf", bufs=1, space="PSUM") as psfpool,
            tc.tile_pool(name="ps4", bufs=1, space="PSUM") as ps4pool,
        ):
            # ---- weights / constants, loaded once (all slices of blob_d)
            w1sb = wpool.tile([60, 5 * 128], F16, tag="w1sb")
            nc.sync.dma_start(
                out=w1sb[:],
                in_=blob_d[O_W1T : O_W1T + 38400].rearrange("(p f) -> p f", p=60),
            )
            w2sb = wpool.tile([96, 9 * 32], F16, tag="w2sb")
            nc.sync.dma_start(
                out=w2sb[:],
                in_=blob_d[O_W2T : O_W2T + 27648].rearrange("(p f) -> p f", p=96),
            )
            # fc1 weights: int4-packed codes -> fp16 (two tensor_scalar unpacks)
            wf1p = wpool.tile([128, FDIM // 2], U8, tag="wf1p")
            nc.sync.dma_start(
                out=wf1p[:],
                in_=blob_d[0 : FDIM // 4 * 128].bitcast(U8).rearrange(
                    "(p f) -> p f", p=128
                ),
            )
            wf1f = wpool.tile([128, FDIM], F16, tag="wf1f")
            wf1v = wf1f.rearrange("p (i two) -> p i two", two=2)
            # (walrus rejects bitwise+arith in one tensor_scalar: split)
            u0 = wpool.tile([128, FDIM // 2], U8, tag="u0")
            nc.vector.tensor_scalar(
                u0[:], wf1p[:], 15, None, op0=mybir.AluOpType.bitwise_and
            )
            nc.vector.tensor_scalar_add(wf1v[:, :, 0], u0[:], -8.0)
            u1 = wpool.tile([128, FDIM // 2], U8, tag="u1")
            nc.vector.tensor_scalar(
                u1[:], wf1p[:], 4, None, op0=mybir.AluOpType.logical_shift_right
            )
            nc.vector.tensor_scalar_add(wf1v[:, :, 1], u1[:], -8.0)
            wf2t = wpool.tile([128, 4], F16, tag="wf2t")
            nc.sync.dma_start(
                out=wf2t[:],
                in_=blob_d[O_WF2 : O_WF2 + 512].rearrange("(p f) -> p f", p=128),
            )
            # biases travel as f16, converted to f32 tiles on chip
            bh = wpool.tile([128, 6], F16, tag="bh")
            nc.sync.dma_start(
                out=bh[:],
                in_=blob_d[O_BIAS : O_BIAS + 768].rearrange("(p f) -> p f", p=128),
            )
            b2r = wpool.tile([128, 1], F32, tag="b2r")
            nc.vector.tensor_copy(b2r[:], bh[:, 0:1])
            bf1c = wpool.tile([128, 1], F32, tag="bf1c")
            nc.vector.tensor_copy(bf1c[:], bh[:, 1:2])
            bf2f = wpool.tile([128, 4], F32, tag="bf2f")
            nc.vector.tensor_copy(bf2f[:], bh[:, 2:6])
            # preload ACT exp LUT so the final Exp carries no table-DMA wait
            warm = wpool.tile([1, 1], F32, tag="warm")
            nc.scalar.activation(
                warm[:], b2r[0:1, :], mybir.ActivationFunctionType.Exp
            )

            for s in range(N_SETS):
                b0 = G * s
                # ---- conv1 im2col: x60[(kd,kh),(j), (do, 14h x 32w)], 15 DMAs
                x60i = xpool.tile([60, 14 * 448], I8, tag="x60i")
                x60p = x60i.rearrange("(t j) f -> t j f", j=G)
                for kd in range(5):
                    for kh in range(3):
                        t = kd * 3 + kh
                        src = AP(
                            x_d,
                            b0 * 16384 + kd * 512 + kh * 32,
                            [[16384, G], [1024, 14], [1, 448]],
                        )
                        nc.sync.dma_start(out=x60p[t], in_=src)
                x60 = xpool.tile([60, 14 * 448], F16, tag="x60")
                nc.vector.tensor_copy(x60[:], x60i[:])

                # ---- conv1: 5 accumulating matmuls (kw) x 7 chunks, 4 img each
                xv = x60.rearrange("p (do ho w) -> p do ho w", do=14, ho=14, w=32)
                c1 = c1pool.tile([128, P1], F16, tag="c1")
                for ch in range(CV1_NCHUNK):
                    ps1 = ps1pool.tile([128, CV1_CHUNK], F32, tag="ps1")
                    for kw in range(5):
                        rhs = xv[:, 2 * ch : 2 * ch + 2, :, kw : kw + 27 : 2]
                        nc.tensor.matmul(
                            ps1[:], w1sb[:, kw * 128 : (kw + 1) * 128], rhs,
                            start=(kw == 0), stop=(kw == 4),
                        )
                    nc.vector.tensor_copy(
                        c1[:, ch * CV1_CHUNK : (ch + 1) * CV1_CHUNK], ps1[:]
                    )

                # ---- conv2 im2col: c96[(kd,ci), (j, d', h, w)] via 12 DMAs
                c96 = c96pool.tile([96, G * C96_FREE], F16, tag="c96")
                c96r = c96.rearrange("q (j f) -> q j f", j=G)
                c1r = c1.rearrange("p (d hw) -> p d hw", d=14, hw=196)
                for kd in range(3):
                    for j in range(G):
                        nc.sync.dma_start(
                            out=c96r[32 * kd : 32 * kd + 32, j],
                            in_=c1r[32 * j : 32 * j + 32, kd : kd + 12, :],
                        )

                # ---- conv2 matmuls (4 img share a PSUM tile) + maxpool
                c96v = c96.rearrange(
                    "q (j d h w) -> q j d h w", j=G, d=D2, h=14, w=14
                )
                pall = ppool.tile([128, POOL_F], F32, tag="pall")
                for t in range(CV2_NCHUNK):
                    ps2 = ps2pool.tile([128, CV2_CHUNK], F32, tag="ps2")
                    for j in range(G):
                        for kk in range(9):
                            kh, kw = kk // 3, kk % 3
                            rhs = c96v[:, j, 2 * t : 2 * t + 2,
                                       kh : kh + D2, kw : kw + D2]
                            nc.tensor.matmul(
                                ps2[32 * j : 32 * j + 32, :],
                                w2sb[:, kk * 32 : (kk + 1) * 32], rhs,
                                start=(kk == 0), stop=(kk == 8),
                                tile_position=(0, 32 * j),
                            )
                    # maxpool 2x2x2 on [128, (2,12,12)] -> [128, 36]
                    # (DVE may read only ONE input from PSUM: copy evens, max odds)
                    pv = ps2.rearrange("p (d h w) -> p d h w", d=2, h=12, w=12)
                    t1 = scratch.tile([128, 144], F32, tag="t1")
                    t1v = t1.rearrange("p (d h w) -> p d h w", d=2, h=12, w=6)
                    nc.vector.tensor_copy(t1v[:], pv[:, :, :, 0::2])
                    nc.vector.tensor_max(t1v[:], t1v[:], pv[:, :, :, 1::2])
                    t2 = scratch.tile([128, 72], F32, tag="t2")
                    t2v = t2.rearrange("p (d h w) -> p d h w", d=2, h=6, w=6)
                    nc.vector.tensor_max(t2v[:], t1v[:, :, 0::2, :], t1v[:, :, 1::2, :])
                    nc.vector.tensor_max(
                        pall[:, t * 36 : (t + 1) * 36], t2[:, 0:36], t2[:, 36:72]
                    )
                # bias b2 (post-pool equivalent) + cast fp16
                psb = scratch.tile([128, POOL_F], F16, tag="psb")
                nc.vector.tensor_scalar_add(psb[:], pall[:], b2r[:])
                # store features [ (j,co), 216 ] -> f_dram[4 imgs, 6912]
                for j in range(G):
                    nc.sync.dma_start(
                        out=f_dram[b0 + j : b0 + j + 1, :],
                        in_=psb[32 * j : 32 * j + 32, :],
                    )

            # ---- fc1: K=6912 in 54 chunks via xbar-transposed loads
            psf = psfpool.tile([128, 128], F32, tag="psf")
            for c in range(FC_NCHUNK):
                fcc = fpool.tile([128, 128], F16, tag="fcc")
                nc.sync.dma_start(
                    out=fcc[:], in_=f_dram[:, 128 * c : 128 * (c + 1)],
                    transpose=True,
                )
                nc.tensor.matmul(
                    psf[:], wf1f[:, 128 * c : 128 * (c + 1)], fcc[:],
                    start=(c == 0), stop=(c == FC_NCHUNK - 1),
                )
            # relu(s1 + bf1/s) -> A1 [128(out_f), 128(img)] fp16
            s1t = fpool.tile([128, 128], F32, tag="s1t")
            nc.vector.tensor_scalar_add(s1t[:], psf[:], bf1c[:])
            a1 = fpool.tile([128, 128], F16, tag="a1")
            nc.vector.tensor_scalar_max(a1[:], s1t[:], 0.0)
            # fc2 (wf2 carries the fc1 quant scale)
            ps4 = ps4pool.tile([128, 4], F32, tag="ps4")
            nc.tensor.matmul(ps4[:], a1[:], wf2t[:], start=True, stop=True)
            s2 = scratch.tile([128, 4], F32, tag="s2")
            nc.vector.tensor_add(s2[:], ps4[:], bf2f[:])
            # softmax over free dim (4)
            nmax = scratch.tile([128, 1], F32, tag="nmax")
            nc.vector.reduce_max(
                out=nmax[:], in_=s2[:], axis=mybir.AxisListType.X, negate=True
            )
            ex = scratch.tile([128, 4], F32, tag="ex")
            esum = scratch.tile([128, 1], F32, tag="esum")
            nc.scalar.activation(
                ex[:], s2[:], mybir.ActivationFunctionType.Exp,
                bias=nmax[:], accum_out=esum[:],
            )
            rec = scratch.tile([128, 1], F32, tag="rec")
            nc.vector.reciprocal(rec[:], esum[:])
            outt = scratch.tile([128, 4], F32, tag="outt")
            nc.vector.tensor_scalar_mul(outt[:], ex[:], rec[:])
            nc.sync.dma_start(out=out_d[:], in_=outt[:])

    nc.compile()
    return nc


_CACHED = {}


def _host_prep(x, w1, b1, w2, b2, wf1, bf1, wf2, bf2):
    q1 = _fake_quant(w1)
    q2 = _fake_quant(w2)
    qf2 = _fake_quant(wf2)
    # fc1 weights as int4 codes; scale folded into wf2 / bf1
    sf1 = float(np.max(np.abs(np.asarray(wf1, np.float32))) / 7.0)
    kf1 = np.clip(
        np.round(np.asarray(wf1, np.float32) / sf1), -7, 7
    ).astype(np.int8)  # [128, 6912]

    # x as int8, per-tensor scale folded into conv1 weights
    Xf = np.asarray(x, np.float32)[:, 0]
    sx = float(np.max(np.abs(Xf)) / 127.0)
    # no clip needed: sx = max|x|/127 bounds rounded values to [-127, 127]
    Xq = Xf * (1.0 / sx)
    np.rint(Xq, out=Xq)
    Xi = Xq.astype(np.int8)

    # conv1 weights: [60, 5*128], block-diag over 4 images, per-kw slices
    # partition order (t, j) with t = kd*3+kh
    w1t = np.zeros((5, 60, 128), np.float32)
    for kw in range(5):
        for kd in range(5):
            for kh in range(3):
                t = kd * 3 + kh
                for j in range(G):
                    w1t[kw, t * G + j, j * 32 : (j + 1) * 32] = q1[:, 0, kd, kh, kw]
    w1t = np.ascontiguousarray(
        (w1t * sx).transpose(1, 0, 2).reshape(60, 5 * 128)
    ).astype(np.float16)

    # conv2 weights: [96=(kd,ci), 9*32=(kk,co)]
    W2T = np.empty((9, 96, 32), np.float32)
    for kh in range(3):
        for kw in range(3):
            for kd in range(3):
                W2T[kh * 3 + kw, kd * 32 : (kd + 1) * 32, :] = q2[:, :, kd, kh, kw].T
    W2T = np.ascontiguousarray(
        W2T.transpose(1, 0, 2).reshape(96, 288)
    ).astype(np.float16)

    # fc1 int4 codes laid out [p, (chunk, out)] with p = feature-within-chunk,
    # then packed two-per-byte (lo nibble = even column, offset +8)
    wf1i = np.ascontiguousarray(
        kf1.T.reshape(FC_NCHUNK, 128, 128).transpose(1, 0, 2).reshape(128, FDIM)
    )
    lo = (wf1i[:, 0::2] + 8).astype(np.uint8)
    hi = (wf1i[:, 1::2] + 8).astype(np.uint8)
    wf1p = (lo | (hi << 4)).astype(np.uint8)

    wf2t = np.ascontiguousarray(qf2.T * sf1).astype(np.float16)  # [128, 4]

    b2p = np.asarray(b2, np.float32) + q2.sum(axis=(2, 3, 4)) @ np.asarray(
        b1, np.float32
    )
    b2r = np.tile(b2p, G)[:, None].astype(np.float32)
    bf1c = (np.asarray(bf1, np.float32) / sf1)[:, None]
    bf2f = np.tile(np.asarray(bf2, np.float32)[None, :], (128, 1))

    blob = np.empty(BLOB_F16, np.float16)
    blob[0:O_W1T] = wf1p.reshape(-1).view(np.float16)
    blob[O_W1T : O_W1T + 38400] = w1t.reshape(-1)
    blob[O_W2T : O_W2T + 27648] = W2T.reshape(-1)
    blob[O_WF2 : O_WF2 + 512] = wf2t.reshape(-1)
    blob[O_BIAS : O_BIAS + 768] = np.concatenate(
        [b2r, bf1c, bf2f], axis=1
    ).astype(np.float16).reshape(-1)
    return Xi, blob


def kernel(x, w1, b1, w2, b2, wf1, bf1, wf2, bf2):
    Xi, blob = _host_prep(x, w1, b1, w2, b2, wf1, bf1, wf2, bf2)
    if "nc" not in _CACHED:
        _CACHED["nc"] = _build_nc()
    nc = _CACHED["nc"]
    in_maps = []
    for c in range(N_CORES):
        in_maps.append({
            "x": Xi[c * B_CORE : (c + 1) * B_CORE],
            "blob": blob,
        })
    res = run_bass_kernel_spmd(nc, in_maps, list(range(N_CORES)))
    outs = [np.asarray(r["out"], np.float32) for r in res.results]
    return np.concatenate(outs, axis=0)
